# revision 6
# baseline (speedup 1.0000x reference)
"""Multi-head dense attention (no softmax) on 8 Trainium2 NeuronCores.

Math (per batch b, head h with head_dim d=64):
    q   = x @ W^T                      # [S, H] projection
    out_h = (q_h x_h^T) x_h            # naive: O(S^2 d) with an SxS temp
          = q_h (x_h^T x_h)            # reassociated: Gram matrix G_h [d, d]
The reassociation is exact (same sum, different order) and collapses the
FLOPs ~5x while removing the SxS intermediate entirely.

Sharding: core c handles batch b = c//2 and head-group hg = c%2 (8 heads,
512 output columns). Cores are fully independent (no collectives).

v3 schedule (v1 baseline 61.9us, v2 59.3us). Trace findings driving v3:
  - ~6.5us framework preamble before any user instruction, input DMA wire
    starts ~8us: structural, shared by every variant.
  - Each dma_start costs ~0.62us on the issuing sequencer. v2's 128KiB
    trigger interleave was trigger-issue-bound (245 GB/s vs the ring's
    323 GB/s): v3 uses >=256KiB triggers so the wire stays fed.
  - PE steady state is ~230ns per N=512 matmul (full 2.4GHz) but the
    first ~3us of busy run at 0.65-1.2GHz: a single back-to-back warmup
    accumulation chain (no pool-slot churn => no sem stalls) spins the
    clock up during the DMA-latency window, sized to finish right as the
    first wT/xT chunks land.
  - PSUM->SBUF drains are split in halves across Vector and Scalar
    (Activation) so drain latency (not throughput) halves; psq drains gate
    the next s-chunk's psum reuse and are emitted before out-stage work.
  - Stores batch per s-chunk (one 512KiB trigger instead of 4) on GpSimd;
    v2's tail serialized 4 triggers at 0.64us each.
  - Gram in fp8e4 DoubleRow (2 s-tiles per instruction, 0.5 cyc/row);
    fp8 for the projection itself was simulated and FAILS the 2e-2 gate
    (3.5e-2), so the big GEMM stays fp16. xn e4m3 Gram noise costs
    rel_err ~7e-3 (vs 0.9e-3 all-fp16), inside the gate with 2.8x margin.

Tensor order: warmup | proj0 | proj1 | gram | out0 | proj2 | out1 |
proj3 | out2 | out3 — each out-stage trails its s-chunk by one proj so
drains complete off the critical path; out3 is the only exposed tail.

Device layout per core:
    xT  [SC*KT*128, 512] f16  x[b]^T blocked (sc,kt)-major, 128KiB/chunk
    xn  [2048, 512]      f8e4 x[b] natural, head-group cols (Gram operand)
    wT  [KT*128, 512]    f16  1024*W_hg^T, k-major (kt blocks contiguous)
    outB [SC*MT*128, 512] f16 out^T blocked (sc,mt); host reassembles
"""

import numpy as np

B, S, H = 4, 2048, 1024
N_HEADS = 16
HD = H // N_HEADS  # 64
N_CORES = 8
MG = H // 2        # 512 output columns per core
P = 128
KT = H // P        # 8 k-tiles
ST = S // P        # 16 s-tiles
MT = MG // P       # 4 m-tiles == head pairs
SC = S // 512      # 4 s-chunks
W_SCALE = 1024.0
N_WARMUP = 12

_NC_CACHE = {}


def _build_nc():
    import concourse.mybir as mybir
    from concourse import bacc
    from concourse.tile import TileContext

    f32 = mybir.dt.float32
    f16 = mybir.dt.float16
    f8e4 = mybir.dt.float8e4
    DR = mybir.MatmulPerfMode.DoubleRow

    nc = bacc.Bacc()
    wT_d = nc.declare_dram_parameter("wT", [KT * P, MG], f16, isOutput=False)
    xT_d = nc.declare_dram_parameter("xT", [SC * KT * P, 512], f16, isOutput=False)
    xn_d = nc.declare_dram_parameter("xn", [S, MG], f8e4, isOutput=False)
    outB_d = nc.declare_dram_parameter("outB", [SC * MT * P, 512], f16, isOutput=True)

    wT_t = wT_d.rearrange("(kt p) m -> p kt m", p=P)             # [128, 8, 512]
    xT_t = xT_d.rearrange("(sc kt p) n -> p sc kt n", sc=SC, kt=KT, p=P)
    xn_t = xn_d.rearrange("(st p) m -> p st m", p=P)             # [128, 16, 512]
    outB_t = outB_d.rearrange("(sc mt p) n -> p sc mt n", sc=SC, mt=MT, p=P)

    with TileContext(nc) as tc:
        with (
            tc.tile_pool(name="big", bufs=1) as big,
            tc.tile_pool(name="gp", bufs=1) as gpool,
            tc.tile_pool(name="stage", bufs=2) as stage,
            tc.tile_pool(name="ps_q", bufs=1, space="PSUM") as ps_q,
            tc.tile_pool(name="ps_g", bufs=2, space="PSUM") as ps_g,
            tc.tile_pool(name="ps_o", bufs=2, space="PSUM") as ps_o,
        ):
            wT_sb = big.tile([P, KT, MG], f16, tag="wT")
            xT_sb = big.tile([P, SC, KT, 512], f16, tag="xT")
            xn_sb = big.tile([P, ST, MG], f8e4, tag="xn")
            q_sb = big.tile([P, MT, S], f16, tag="q")

            # ---- Warmup: one back-to-back accumulation chain (same psum
            # tile, same engine => no semaphores) spins the PE p-state up
            # during the initial DMA latency window.
            wu_sb = gpool.tile([P, 512], f16, tag="wu", name="wu_sb")
            nc.vector.memset(wu_sb, 0.0)
            # Gram block-diagonal lhsT tiles; memset early (DVE is idle).
            gbd = []
            for p_i in range(MT):
                g = gpool.tile([P, P], f16, tag=f"g{p_i}", name=f"g{p_i}")
                nc.vector.memset(g, 0.0)
                gbd.append(g)
            wu_ps = ps_o.tile([P, 256], f32, tag="pso", name="wu_ps")
            for i in range(N_WARMUP):
                nc.tensor.matmul(
                    wu_ps,
                    lhsT=wu_sb[:, 0:P],
                    rhs=wu_sb[:, 0:256],
                    start=(i == 0),
                    stop=(i == N_WARMUP - 1),
                )

            # ---- Input DMA ring (Sync engine), exact consumption order.
            # First k-tile ships alone (fast start), the rest in >=256KiB
            # chunks so the wire (323 GB/s) outruns trigger issue (0.62us).
            nc.sync.dma_start(out=wT_sb[:, 0], in_=wT_t[:, 0])
            nc.sync.dma_start(out=xT_sb[:, 0, 0], in_=xT_t[:, 0, 0])
            for a, z in ((1, 3), (3, 5), (5, 8)):
                nc.sync.dma_start(out=wT_sb[:, a:z], in_=wT_t[:, a:z])
                nc.sync.dma_start(out=xT_sb[:, 0, a:z], in_=xT_t[:, 0, a:z])
            for i in range(4):
                nc.sync.dma_start(
                    out=xT_sb[:, 1, 2 * i:2 * i + 2], in_=xT_t[:, 1, 2 * i:2 * i + 2]
                )
            nc.sync.dma_start(out=xn_sb[:, 0:8], in_=xn_t[:, 0:8])
            nc.sync.dma_start(out=xn_sb[:, 8:16], in_=xn_t[:, 8:16])
            nc.sync.dma_start(out=xT_sb[:, 2, 0:4], in_=xT_t[:, 2, 0:4])
            nc.sync.dma_start(out=xT_sb[:, 2, 4:8], in_=xT_t[:, 2, 4:8])
            nc.sync.dma_start(out=xT_sb[:, 3, 0:4], in_=xT_t[:, 3, 0:4])
            nc.sync.dma_start(out=xT_sb[:, 3, 4:8], in_=xT_t[:, 3, 4:8])

            def proj(sc):
                psqs = [
                    ps_q.tile([P, 512], f32, tag=f"psq{mt}", name=f"psq{sc}_{mt}")
                    for mt in range(MT)
                ]
                for kt in range(KT):
                    for mt in range(MT):
                        nc.tensor.matmul(
                            psqs[mt],
                            lhsT=wT_sb[:, kt, mt * P:(mt + 1) * P],
                            rhs=xT_sb[:, sc, kt],
                            start=(kt == 0),
                            stop=(kt == KT - 1),
                        )
                # Half-drains: DVE takes the low half, Activation the high
                # half; latency per psum bank halves and both engines stay
                # evenly loaded.
                for mt in range(MT):
                    lo = q_sb[:, mt, sc * 512:sc * 512 + 256]
                    hi = q_sb[:, mt, sc * 512 + 256:(sc + 1) * 512]
                    nc.vector.tensor_copy(out=lo, in_=psqs[mt][:, 0:256])
                    nc.scalar.copy(out=hi, in_=psqs[mt][:, 256:512])

            def gram():
                for p_i in range(MT):
                    psg = ps_g.tile([P, P], f32, tag="psg", name=f"psg{p_i}")
                    xp = xn_sb[:, :, p_i * P:(p_i + 1) * P]
                    for i in range(ST // 2):
                        nc.tensor.matmul(
                            psg,
                            lhsT=xp[:, 2 * i:2 * i + 2],
                            rhs=xp[:, 2 * i:2 * i + 2],
                            start=(i == 0),
                            stop=(i == ST // 2 - 1),
                            perf_mode=DR,
                        )
                    nc.vector.tensor_scalar_mul(
                        out=gbd[p_i][0:HD, 0:HD],
                        in0=psg[0:HD, 0:HD],
                        scalar1=1.0 / W_SCALE,
                    )
                    nc.scalar.mul(
                        gbd[p_i][HD:P, HD:P], psg[HD:P, HD:P], 1.0 / W_SCALE
                    )

            def out_stage(sc):
                ot = stage.tile([P, MT, 512], f16, tag="ot", name=f"ot{sc}")
                for mt in range(MT):
                    pso = ps_o.tile([P, 512], f32, tag="pso", name=f"pso{sc}_{mt}")
                    nc.tensor.matmul(
                        pso,
                        lhsT=gbd[mt],
                        rhs=q_sb[:, mt, sc * 512:(sc + 1) * 512],
                        start=True,
                        stop=True,
                    )
                    nc.vector.tensor_copy(out=ot[:, mt, 0:256], in_=pso[:, 0:256])
                    nc.scalar.copy(out=ot[:, mt, 256:512], in_=pso[:, 256:512])
                nc.gpsimd.dma_start(out=outB_t[:, sc], in_=ot)

            proj(0)
            proj(1)
            gram()
            out_stage(0)
            proj(2)
            out_stage(1)
            proj(3)
            out_stage(2)
            out_stage(3)
    nc.compile()
    return nc


def _get_nc():
    if "nc" not in _NC_CACHE:
        _NC_CACHE["nc"] = _build_nc()
    return _NC_CACHE["nc"]


def make_in_maps(hidden_states, queries_weight):
    import ml_dtypes

    f8e4 = ml_dtypes.float8_e4m3
    hs = np.ascontiguousarray(np.asarray(hidden_states, dtype=np.float32))
    w = np.ascontiguousarray(np.asarray(queries_weight, dtype=np.float32))
    in_maps = []
    for c in range(N_CORES):
        b, hg = divmod(c, 2)
        xb = hs[b]
        xT = np.ascontiguousarray(xb.T)  # [1024, 2048]
        in_maps.append({
            # blocked (sc, kt): rows kt*128..+128, cols sc*512..+512 contiguous
            "xT": np.ascontiguousarray(
                xT.reshape(KT, P, SC, 512).transpose(2, 0, 1, 3).reshape(
                    SC * KT * P, 512
                )
            ).astype(np.float16),
            "xn": np.ascontiguousarray(
                xb[:, hg * MG:(hg + 1) * MG]
            ).astype(f8e4),
            "wT": np.ascontiguousarray(
                w[hg * MG:(hg + 1) * MG, :].T * W_SCALE
            ).astype(np.float16),
        })
    return in_maps


def assemble_output(results):
    out = np.empty((B, S, H), dtype=np.float32)
    for c in range(N_CORES):
        b, hg = divmod(c, 2)
        r = np.asarray(results[c]["outB"])  # [SC*MT*P, 512] f16
        out[b, :, hg * MG:(hg + 1) * MG] = (
            r.reshape(SC, MT, P, 512).transpose(0, 3, 1, 2).reshape(S, MG)
        ).astype(np.float32)
    return out


def kernel(hidden_states, queries_weight):
    from concourse.bass_utils import run_bass_kernel_spmd

    in_maps = make_in_maps(hidden_states, queries_weight)
    res = run_bass_kernel_spmd(
        _get_nc(), in_maps, core_ids=list(range(N_CORES))
    ).results
    return assemble_output(res)


if __name__ == "__main__":
    x = np.random.randn(B, S, H).astype(np.float32)
    w = np.random.randn(H, H).astype(np.float32) * 1e-4
    out = kernel(x, w)
    print(out.shape, out.dtype)


# revision 7
# speedup vs baseline: 1.0106x; 1.0106x over previous
"""Multi-head dense attention (no softmax) on 8 Trainium2 NeuronCores.

Math (per batch b, head h with head_dim d=64):
    q   = x @ W^T                      # [S, H] projection
    out_h = (q_h x_h^T) x_h            # naive: O(S^2 d) with an SxS temp
          = q_h (x_h^T x_h)            # reassociated: Gram matrix G_h [d, d]
The reassociation is exact (same sum, different order) and collapses the
FLOPs ~5x while removing the SxS intermediate entirely.

Sharding: core c handles batch b = c//2 and head-group hg = c%2 (8 heads,
512 output columns). Cores are fully independent (no collectives).

v4 (v1 61.9us, v2 59.3us, v3 60.6us). Trace findings driving v4:
  - ~6.5us framework preamble + ~1.6us trigger->wire latency are fixed;
    first input bytes land ~8.3us, so the s-chunk-0 projection is wire-
    paced: xT ships as fp8 e3m4 (half the bytes => sc0 compute-bound
    sooner, whole input stream done ~21us instead of 29). wT stays fp16
    (wT-as-e3m4 simulated at 1.71e-2 - too close to the 2e-2 gate).
    Projection matmuls are then mixed-dtype (f16 lhsT x f8e3 rhs), same
    1 cyc/row speed; e3m4's 4 mantissa bits keep rel_err ~1.4e-2 (sim).
  - The Activation-function table load (1.28us) was lazily inserted
    before the FIRST Activation drain at t~20, stalling the psq
    turnaround: a dummy scalar copy right after the preamble forces the
    load into the idle warmup window.
  - v3's Gram serialized on a 2-slot PSUM ring (pair p+2 waited pair p's
    drains): all 4 pair-Grams now accumulate in ONE psum bank as
    [128, 4, 128] slices (start/stop zero/track only the addresses each
    chain writes), freeing a bank to double-buffer psq0.
  - psq drains are emitted mt1,mt2,mt3,mt0 (mt0 is double-buffered, so
    the next s-chunk's kt0 matmuls hit no-wait mt0 first while mt1-3
    drain in parallel halves on Vector+Activation).
  - The last store splits into two half-triggers so its wire time
    overlaps the final drains instead of serializing after them.
  - fp8 e4m3 for the projection/out-stage operands was simulated and
    FAILS the gate (2.5-3.5e-2); the big GEMM stays fp16-speed.

Tensor order: warmup | proj0 | proj1 | gram | out0 | proj2 | out1 |
proj3 | out2 | out3 - each out-stage trails its s-chunk by one proj so
drains complete off the critical path; out3 is the only exposed tail.

Device layout per core:
    xT  [SC*KT*128, 512] f8e3 x[b]^T blocked (sc,kt)-major, 64KiB/chunk
    xn  [2048, 512]      f8e4 x[b] natural, head-group cols (Gram operand)
    wT  [KT*128, 512]    f16  1024*W_hg^T, k-major (kt blocks contiguous)
    outB [SC*MT*128, 512] f16 out^T blocked (sc,mt); host reassembles
"""

import numpy as np

B, S, H = 4, 2048, 1024
N_HEADS = 16
HD = H // N_HEADS  # 64
N_CORES = 8
MG = H // 2        # 512 output columns per core
P = 128
KT = H // P        # 8 k-tiles
ST = S // P        # 16 s-tiles
MT = MG // P       # 4 m-tiles == head pairs
SC = S // 512      # 4 s-chunks
W_SCALE = 1024.0
N_WARMUP = 10

_NC_CACHE = {}


def _build_nc():
    import concourse.mybir as mybir
    from concourse import bacc
    from concourse.tile import TileContext

    f32 = mybir.dt.float32
    f16 = mybir.dt.float16
    f8e4 = mybir.dt.float8e4
    f8e3 = mybir.dt.float8e3
    DR = mybir.MatmulPerfMode.DoubleRow

    nc = bacc.Bacc()
    wT_d = nc.declare_dram_parameter("wT", [KT * P, MG], f16, isOutput=False)
    xT_d = nc.declare_dram_parameter("xT", [SC * KT * P, 512], f8e3, isOutput=False)
    xn_d = nc.declare_dram_parameter("xn", [S, MG], f8e4, isOutput=False)
    outB_d = nc.declare_dram_parameter("outB", [SC * MT * P, 512], f16, isOutput=True)

    wT_t = wT_d.rearrange("(kt p) m -> p kt m", p=P)             # [128, 8, 512]
    xT_t = xT_d.rearrange("(sc kt p) n -> p sc kt n", sc=SC, kt=KT, p=P)
    xn_t = xn_d.rearrange("(st p) m -> p st m", p=P)             # [128, 16, 512]
    outB_t = outB_d.rearrange("(sc mt p) n -> p sc mt n", sc=SC, mt=MT, p=P)

    with TileContext(nc) as tc:
        with (
            tc.tile_pool(name="big", bufs=1) as big,
            tc.tile_pool(name="gp", bufs=1) as gpool,
            tc.tile_pool(name="stage", bufs=2) as stage,
            tc.tile_pool(name="ps_q0", bufs=2, space="PSUM") as ps_q0,
            tc.tile_pool(name="ps_q", bufs=1, space="PSUM") as ps_q,
            tc.tile_pool(name="ps_g", bufs=1, space="PSUM") as ps_g,
            tc.tile_pool(name="ps_o", bufs=2, space="PSUM") as ps_o,
        ):
            wT_sb = big.tile([P, KT, MG], f16, tag="wT")
            xT_sb = big.tile([P, SC, KT, 512], f8e3, tag="xT")
            xn_sb = big.tile([P, ST, MG], f8e4, tag="xn")
            q_sb = big.tile([P, MT, S], f16, tag="q")

            # ---- Warmup: one back-to-back accumulation chain (same psum
            # tile, same engine => no semaphores) spins the PE p-state up
            # during the initial DMA latency window. The scalar-engine copy
            # forces the lazy ACT_TABLE_LOAD into this idle window too.
            wu_sb = gpool.tile([P, 512], f16, tag="wu", name="wu_sb")
            nc.vector.memset(wu_sb, 0.0)
            nc.scalar.copy(out=wu_sb[:, 256:264], in_=wu_sb[:, 0:8])
            gbd = []
            for p_i in range(MT):
                g = gpool.tile([P, P], f16, tag=f"g{p_i}", name=f"g{p_i}")
                nc.vector.memset(g, 0.0)
                gbd.append(g)
            wu_ps = ps_o.tile([P, 256], f32, tag="pso", name="wu_ps")
            for i in range(N_WARMUP):
                nc.tensor.matmul(
                    wu_ps,
                    lhsT=wu_sb[:, 0:P],
                    rhs=wu_sb[:, 0:256],
                    start=(i == 0),
                    stop=(i == N_WARMUP - 1),
                )

            # ---- Input DMA ring (Sync engine), exact consumption order,
            # >=192KiB per trigger after the first pair so the wire
            # (~300GB/s) outruns trigger issue (~0.65us each).
            nc.sync.dma_start(out=wT_sb[:, 0], in_=wT_t[:, 0])
            nc.sync.dma_start(out=xT_sb[:, 0, 0], in_=xT_t[:, 0, 0])
            for a, z in ((1, 3), (3, 5), (5, 8)):
                nc.sync.dma_start(out=wT_sb[:, a:z], in_=wT_t[:, a:z])
                nc.sync.dma_start(out=xT_sb[:, 0, a:z], in_=xT_t[:, 0, a:z])
            nc.sync.dma_start(out=xT_sb[:, 1, 0:4], in_=xT_t[:, 1, 0:4])
            nc.sync.dma_start(out=xT_sb[:, 1, 4:8], in_=xT_t[:, 1, 4:8])
            nc.sync.dma_start(out=xn_sb[:, 0:8], in_=xn_t[:, 0:8])
            nc.sync.dma_start(out=xn_sb[:, 8:16], in_=xn_t[:, 8:16])
            nc.sync.dma_start(out=xT_sb[:, 2], in_=xT_t[:, 2])
            nc.sync.dma_start(out=xT_sb[:, 3], in_=xT_t[:, 3])

            def proj(sc):
                psqs = [
                    (ps_q0 if mt == 0 else ps_q).tile(
                        [P, 512], f32, tag=f"psq{mt}", name=f"psq{sc}_{mt}"
                    )
                    for mt in range(MT)
                ]
                for kt in range(KT):
                    for mt in range(MT):
                        nc.tensor.matmul(
                            psqs[mt],
                            lhsT=wT_sb[:, kt, mt * P:(mt + 1) * P],
                            rhs=xT_sb[:, sc, kt],
                            start=(kt == 0),
                            stop=(kt == KT - 1),
                        )
                # Half-drains on DVE+Act; mt0 last (it is double-buffered so
                # the next s-chunk never waits on it).
                for mt in (1, 2, 3, 0):
                    lo = q_sb[:, mt, sc * 512:sc * 512 + 256]
                    hi = q_sb[:, mt, sc * 512 + 256:(sc + 1) * 512]
                    nc.vector.tensor_copy(out=lo, in_=psqs[mt][:, 0:256])
                    nc.scalar.copy(out=hi, in_=psqs[mt][:, 256:512])

            def gram():
                # All 4 pair-Grams accumulate in one psum bank; start/stop
                # zero/track only the 128x128 slice each chain writes.
                psg = ps_g.tile([P, MT, P], f32, tag="psg", name="psg")
                for p_i in range(MT):
                    xp = xn_sb[:, :, p_i * P:(p_i + 1) * P]
                    for i in range(ST // 2):
                        nc.tensor.matmul(
                            psg[:, p_i],
                            lhsT=xp[:, 2 * i:2 * i + 2],
                            rhs=xp[:, 2 * i:2 * i + 2],
                            start=(i == 0),
                            stop=(i == ST // 2 - 1),
                            perf_mode=DR,
                        )
                    nc.vector.tensor_scalar_mul(
                        out=gbd[p_i][0:HD, 0:HD],
                        in0=psg[0:HD, p_i, 0:HD],
                        scalar1=1.0 / W_SCALE,
                    )
                    nc.scalar.mul(
                        gbd[p_i][HD:P, HD:P], psg[HD:P, p_i, HD:P], 1.0 / W_SCALE
                    )

            def out_stage(sc):
                ot = stage.tile([P, MT, 512], f16, tag="ot", name=f"ot{sc}")
                for mt in range(MT):
                    pso = ps_o.tile([P, 512], f32, tag="pso", name=f"pso{sc}_{mt}")
                    nc.tensor.matmul(
                        pso,
                        lhsT=gbd[mt],
                        rhs=q_sb[:, mt, sc * 512:(sc + 1) * 512],
                        start=True,
                        stop=True,
                    )
                    nc.vector.tensor_copy(out=ot[:, mt, 0:256], in_=pso[:, 0:256])
                    nc.scalar.copy(out=ot[:, mt, 256:512], in_=pso[:, 256:512])
                    if sc == SC - 1 and mt == 1:
                        nc.gpsimd.dma_start(
                            out=outB_t[:, sc, 0:2], in_=ot[:, 0:2]
                        )
                if sc == SC - 1:
                    nc.gpsimd.dma_start(out=outB_t[:, sc, 2:4], in_=ot[:, 2:4])
                else:
                    nc.gpsimd.dma_start(out=outB_t[:, sc], in_=ot)

            proj(0)
            proj(1)
            gram()
            out_stage(0)
            proj(2)
            out_stage(1)
            proj(3)
            out_stage(2)
            out_stage(3)
    nc.compile()
    return nc


def _get_nc():
    if "nc" not in _NC_CACHE:
        _NC_CACHE["nc"] = _build_nc()
    return _NC_CACHE["nc"]


def make_in_maps(hidden_states, queries_weight):
    import ml_dtypes

    f8e4 = ml_dtypes.float8_e4m3
    f8e3 = ml_dtypes.float8_e3m4
    hs = np.ascontiguousarray(np.asarray(hidden_states, dtype=np.float32))
    w = np.ascontiguousarray(np.asarray(queries_weight, dtype=np.float32))
    in_maps = []
    for c in range(N_CORES):
        b, hg = divmod(c, 2)
        xb = hs[b]
        xT = np.ascontiguousarray(xb.T)  # [1024, 2048]
        in_maps.append({
            # blocked (sc, kt): rows kt*128..+128, cols sc*512..+512 contiguous
            "xT": np.ascontiguousarray(
                xT.reshape(KT, P, SC, 512).transpose(2, 0, 1, 3).reshape(
                    SC * KT * P, 512
                )
            ).astype(f8e3),
            "xn": np.ascontiguousarray(
                xb[:, hg * MG:(hg + 1) * MG]
            ).astype(f8e4),
            "wT": np.ascontiguousarray(
                w[hg * MG:(hg + 1) * MG, :].T * W_SCALE
            ).astype(np.float16),
        })
    return in_maps


def assemble_output(results):
    out = np.empty((B, S, H), dtype=np.float32)
    for c in range(N_CORES):
        b, hg = divmod(c, 2)
        r = np.asarray(results[c]["outB"])  # [SC*MT*P, 512] f16
        out[b, :, hg * MG:(hg + 1) * MG] = (
            r.reshape(SC, MT, P, 512).transpose(0, 3, 1, 2).reshape(S, MG)
        ).astype(np.float32)
    return out


def kernel(hidden_states, queries_weight):
    from concourse.bass_utils import run_bass_kernel_spmd

    in_maps = make_in_maps(hidden_states, queries_weight)
    res = run_bass_kernel_spmd(
        _get_nc(), in_maps, core_ids=list(range(N_CORES))
    ).results
    return assemble_output(res)


if __name__ == "__main__":
    x = np.random.randn(B, S, H).astype(np.float32)
    w = np.random.randn(H, H).astype(np.float32) * 1e-4
    out = kernel(x, w)
    print(out.shape, out.dtype)


# revision 8
# speedup vs baseline: 1.1239x; 1.1120x over previous
"""Multi-head dense attention (no softmax) on 8 Trainium2 NeuronCores.

Math (per batch b, head h with head_dim d=64):
    q   = x @ W^T                      # [S, H] projection
    out_h = (q_h x_h^T) x_h            # naive: O(S^2 d) with an SxS temp
          = q_h (x_h^T x_h)            # reassociated: Gram matrix G_h [d, d]
The reassociation is exact (same sum, different order) and collapses the
FLOPs ~5x while removing the SxS intermediate entirely.

Sharding: core c handles batch b = c//2 and head-group hg = c%2 (8 heads,
512 output columns). Cores are fully independent (no collectives).

v5 (v1 61.9us, v2 59.3, v3 60.6, v4 60.0). Trace findings driving v5:
  - v4's DMA ran 250 GB/s (vs the ring's 320+): fp8 shrank per-partition
    lines to 512B and the DMA is packet-rate-bound. All HBM layouts are
    now partition-outer so every transfer reads 1-4KiB contiguous per
    partition row.
  - The out-stage PSUM ring (bufs=2) serialized: matmul mt2 waited on
    mt0's drains, stretching every out-stage to ~2.5us. pso now has 3
    banks (the Gram pool was folded into the same ring, freeing its
    bank), and the ot staging ring has 4 slots so stores never gate
    drains.
  - out2 is emitted before proj3, leaving only out3 in the tail; the
    last store splits into two half-triggers so wire time overlaps the
    final drains.
  - psq drain order is (1,2,3,0) before a following projection (mt0 is
    double-buffered; mt1-3 drain first so the next s-chunk's kt0 hits
    them ready) and (0,1,2,3) before an out-stage (consumption order).
  - Kept from v4: xT as fp8 e3m4 (mixed f16xf8e3 matmul verified on HW:
    rel_err 1.354e-2 = exactly the numpy simulation; fp8 e4m3 anywhere
    in the projection FAILS the 2e-2 gate at 2.5-3.5e-2), e4m3 DoubleRow
    Gram, warmup chain + early ACT-table preload, drains split in halves
    across Vector+Activation, all input DMA on one Sync-ring in
    consumption order.

Tensor order: warmup | proj0 | proj1 | gram | out0 | proj2 | out1 |
out2 | proj3 | out3.

Device layout per core (all partition-outer):
    xT  [SC*128, KT*512] f8e3  xT[k, s] blocked: row sc*128+p holds all
                               kt chunks for that (sc, p)
    xn  [128, ST*512]    f8e4  row p holds all st chunks
    wT  [128, KT*512]    f16   1024*W_hg^T, row p = all kt chunks
    outB [128, SC*MT*512] f16  row p = out^T chunks; host reassembles
"""

import numpy as np

B, S, H = 4, 2048, 1024
N_HEADS = 16
HD = H // N_HEADS  # 64
N_CORES = 8
MG = H // 2        # 512 output columns per core
P = 128
KT = H // P        # 8 k-tiles
ST = S // P        # 16 s-tiles
MT = MG // P       # 4 m-tiles == head pairs
SC = S // 512      # 4 s-chunks
W_SCALE = 1024.0
N_WARMUP = 10

_NC_CACHE = {}


def _build_nc():
    import concourse.mybir as mybir
    from concourse import bacc
    from concourse.tile import TileContext

    f32 = mybir.dt.float32
    f16 = mybir.dt.float16
    f8e4 = mybir.dt.float8e4
    f8e3 = mybir.dt.float8e3
    DR = mybir.MatmulPerfMode.DoubleRow

    nc = bacc.Bacc()
    wT_d = nc.declare_dram_parameter("wT", [P, KT * MG], f16, isOutput=False)
    xT_d = nc.declare_dram_parameter("xT", [SC * P, KT * 512], f8e3, isOutput=False)
    xn_d = nc.declare_dram_parameter("xn", [P, ST * MG], f8e4, isOutput=False)
    outB_d = nc.declare_dram_parameter(
        "outB", [P, SC * MT * 512], f16, isOutput=True
    )

    wT_t = wT_d.rearrange("p (kt m) -> p kt m", kt=KT)           # [128, 8, 512]
    xT_t = xT_d.rearrange("(sc p) (kt n) -> p sc kt n", sc=SC, kt=KT)
    xn_t = xn_d.rearrange("p (st m) -> p st m", st=ST)           # [128, 16, 512]
    outB_t = outB_d.rearrange("p (sc mt n) -> p sc mt n", sc=SC, mt=MT)

    with TileContext(nc) as tc:
        with (
            tc.tile_pool(name="big", bufs=1) as big,
            tc.tile_pool(name="gp", bufs=1) as gpool,
            tc.tile_pool(name="stage", bufs=4) as stage,
            tc.tile_pool(name="ps_q0", bufs=2, space="PSUM") as ps_q0,
            tc.tile_pool(name="ps_q", bufs=1, space="PSUM") as ps_q,
            tc.tile_pool(name="ps_o", bufs=3, space="PSUM") as ps_o,
        ):
            wT_sb = big.tile([P, KT, MG], f16, tag="wT")
            xT_sb = big.tile([P, SC, KT, 512], f8e3, tag="xT")
            xn_sb = big.tile([P, ST, MG], f8e4, tag="xn")
            q_sb = big.tile([P, MT, S], f16, tag="q")

            # ---- Warmup: one back-to-back accumulation chain (same psum
            # tile, same engine => no semaphores) spins the PE p-state up
            # during the initial DMA latency window. The scalar-engine copy
            # forces the lazy ACT_TABLE_LOAD into this idle window too.
            wu_sb = gpool.tile([P, 512], f16, tag="wu", name="wu_sb")
            nc.vector.memset(wu_sb, 0.0)
            nc.scalar.copy(out=wu_sb[:, 256:264], in_=wu_sb[:, 0:8])
            gbd = []
            for p_i in range(MT):
                g = gpool.tile([P, P], f16, tag=f"g{p_i}", name=f"g{p_i}")
                nc.vector.memset(g, 0.0)
                gbd.append(g)
            wu_ps = ps_o.tile([P, 256], f32, tag="pso", name="wu_ps")
            for i in range(N_WARMUP):
                nc.tensor.matmul(
                    wu_ps,
                    lhsT=wu_sb[:, 0:P],
                    rhs=wu_sb[:, 0:256],
                    start=(i == 0),
                    stop=(i == N_WARMUP - 1),
                )

            # ---- Input DMA ring (Sync engine), exact consumption order,
            # >=192KiB per trigger after the first pair so the wire
            # (~300GB/s) outruns trigger issue (~0.65us each).
            nc.sync.dma_start(out=wT_sb[:, 0], in_=wT_t[:, 0])
            nc.sync.dma_start(out=xT_sb[:, 0, 0], in_=xT_t[:, 0, 0])
            for a, z in ((1, 3), (3, 5), (5, 8)):
                nc.sync.dma_start(out=wT_sb[:, a:z], in_=wT_t[:, a:z])
                nc.sync.dma_start(out=xT_sb[:, 0, a:z], in_=xT_t[:, 0, a:z])
            nc.sync.dma_start(out=xT_sb[:, 1, 0:4], in_=xT_t[:, 1, 0:4])
            nc.sync.dma_start(out=xT_sb[:, 1, 4:8], in_=xT_t[:, 1, 4:8])
            nc.sync.dma_start(out=xn_sb[:, 0:8], in_=xn_t[:, 0:8])
            nc.sync.dma_start(out=xn_sb[:, 8:16], in_=xn_t[:, 8:16])
            nc.sync.dma_start(out=xT_sb[:, 2], in_=xT_t[:, 2])
            nc.sync.dma_start(out=xT_sb[:, 3], in_=xT_t[:, 3])

            def proj(sc, drain_order):
                psqs = [
                    (ps_q0 if mt == 0 else ps_q).tile(
                        [P, 512], f32, tag=f"psq{mt}", name=f"psq{sc}_{mt}"
                    )
                    for mt in range(MT)
                ]
                for kt in range(KT):
                    for mt in range(MT):
                        nc.tensor.matmul(
                            psqs[mt],
                            lhsT=wT_sb[:, kt, mt * P:(mt + 1) * P],
                            rhs=xT_sb[:, sc, kt],
                            start=(kt == 0),
                            stop=(kt == KT - 1),
                        )
                for mt in drain_order:
                    lo = q_sb[:, mt, sc * 512:sc * 512 + 256]
                    hi = q_sb[:, mt, sc * 512 + 256:(sc + 1) * 512]
                    nc.vector.tensor_copy(out=lo, in_=psqs[mt][:, 0:256])
                    nc.scalar.copy(out=hi, in_=psqs[mt][:, 256:512])

            def gram():
                # Pair-Grams cycle the 3-deep pso ring; each pair's two
                # scale-copies run on DVE/Act while the next pair matmuls.
                for p_i in range(MT):
                    psg = ps_o.tile([P, P], f32, tag="pso", name=f"psg{p_i}")
                    xp = xn_sb[:, :, p_i * P:(p_i + 1) * P]
                    for i in range(ST // 2):
                        nc.tensor.matmul(
                            psg,
                            lhsT=xp[:, 2 * i:2 * i + 2],
                            rhs=xp[:, 2 * i:2 * i + 2],
                            start=(i == 0),
                            stop=(i == ST // 2 - 1),
                            perf_mode=DR,
                        )
                    nc.vector.tensor_scalar_mul(
                        out=gbd[p_i][0:HD, 0:HD],
                        in0=psg[0:HD, 0:HD],
                        scalar1=1.0 / W_SCALE,
                    )
                    nc.scalar.mul(
                        gbd[p_i][HD:P, HD:P], psg[HD:P, HD:P], 1.0 / W_SCALE
                    )

            def out_stage(sc):
                ot = stage.tile([P, MT, 512], f16, tag="ot", name=f"ot{sc}")
                for mt in range(MT):
                    pso = ps_o.tile([P, 512], f32, tag="pso", name=f"pso{sc}_{mt}")
                    nc.tensor.matmul(
                        pso,
                        lhsT=gbd[mt],
                        rhs=q_sb[:, mt, sc * 512:(sc + 1) * 512],
                        start=True,
                        stop=True,
                    )
                    nc.vector.tensor_copy(out=ot[:, mt, 0:256], in_=pso[:, 0:256])
                    nc.scalar.copy(out=ot[:, mt, 256:512], in_=pso[:, 256:512])
                    if sc == SC - 1 and mt == 1:
                        nc.gpsimd.dma_start(
                            out=outB_t[:, sc, 0:2], in_=ot[:, 0:2]
                        )
                if sc == SC - 1:
                    nc.gpsimd.dma_start(out=outB_t[:, sc, 2:4], in_=ot[:, 2:4])
                else:
                    nc.gpsimd.dma_start(out=outB_t[:, sc], in_=ot)

            proj(0, (1, 2, 3, 0))
            proj(1, (1, 2, 3, 0))
            gram()
            out_stage(0)
            proj(2, (0, 1, 2, 3))
            out_stage(1)
            out_stage(2)
            proj(3, (0, 1, 2, 3))
            out_stage(3)
    nc.compile()
    return nc


def _get_nc():
    if "nc" not in _NC_CACHE:
        _NC_CACHE["nc"] = _build_nc()
    return _NC_CACHE["nc"]


def make_in_maps(hidden_states, queries_weight):
    import ml_dtypes

    f8e4 = ml_dtypes.float8_e4m3
    f8e3 = ml_dtypes.float8_e3m4
    hs = np.ascontiguousarray(np.asarray(hidden_states, dtype=np.float32))
    w = np.ascontiguousarray(np.asarray(queries_weight, dtype=np.float32))
    in_maps = []
    for c in range(N_CORES):
        b, hg = divmod(c, 2)
        xb = hs[b]
        xT = np.ascontiguousarray(xb.T)  # [1024, 2048]
        in_maps.append({
            # row sc*128+p holds kt-major chunks: [SC*P, KT*512]
            "xT": np.ascontiguousarray(
                xT.reshape(KT, P, SC, 512).transpose(2, 1, 0, 3).reshape(
                    SC * P, KT * 512
                )
            ).astype(f8e3),
            # row p holds st-major chunks: [P, ST*MG]
            "xn": np.ascontiguousarray(
                xb[:, hg * MG:(hg + 1) * MG]
                .reshape(ST, P, MG).transpose(1, 0, 2).reshape(P, ST * MG)
            ).astype(f8e4),
            # row p holds kt-major chunks: [P, KT*MG]
            "wT": np.ascontiguousarray(
                (w[hg * MG:(hg + 1) * MG, :].T * W_SCALE)
                .reshape(KT, P, MG).transpose(1, 0, 2).reshape(P, KT * MG)
            ).astype(np.float16),
        })
    return in_maps


def assemble_output(results):
    out = np.empty((B, S, H), dtype=np.float32)
    for c in range(N_CORES):
        b, hg = divmod(c, 2)
        r = np.asarray(results[c]["outB"])  # [P, SC*MT*512] f16
        out[b, :, hg * MG:(hg + 1) * MG] = (
            r.reshape(P, SC, MT, 512).transpose(1, 3, 2, 0).reshape(S, MG)
        ).astype(np.float32)
    return out


def kernel(hidden_states, queries_weight):
    from concourse.bass_utils import run_bass_kernel_spmd

    in_maps = make_in_maps(hidden_states, queries_weight)
    res = run_bass_kernel_spmd(
        _get_nc(), in_maps, core_ids=list(range(N_CORES))
    ).results
    return assemble_output(res)


if __name__ == "__main__":
    x = np.random.randn(B, S, H).astype(np.float32)
    w = np.random.randn(H, H).astype(np.float32) * 1e-4
    out = kernel(x, w)
    print(out.shape, out.dtype)


# revision 11
# speedup vs baseline: 1.1300x; 1.0055x over previous
"""Multi-head dense attention (no softmax) on 8 Trainium2 NeuronCores.

Math (per batch b, head h with head_dim d=64):
    q   = x @ W^T                      # [S, H] projection
    out_h = (q_h x_h^T) x_h            # naive: O(S^2 d) with an SxS temp
          = q_h (x_h^T x_h)            # reassociated: Gram matrix G_h [d, d]
The reassociation is exact (same sum, different order) and collapses the
FLOPs ~5x while removing the SxS intermediate entirely.

Sharding: core c handles batch b = c//2 and head-group hg = c%2 (8 heads,
512 output columns). Cores are fully independent (no collectives).

v5 (v1 61.9us, v2 59.3, v3 60.6, v4 60.0). Trace findings driving v5:
  - v4's DMA ran 250 GB/s (vs the ring's 320+): fp8 shrank per-partition
    lines to 512B and the DMA is packet-rate-bound. All HBM layouts are
    now partition-outer so every transfer reads 1-4KiB contiguous per
    partition row.
  - The out-stage PSUM ring (bufs=2) serialized: matmul mt2 waited on
    mt0's drains, stretching every out-stage to ~2.5us. pso now has 3
    banks (the Gram pool was folded into the same ring, freeing its
    bank), and the ot staging ring has 4 slots so stores never gate
    drains.
  - out2 is emitted before proj3, leaving only out3 in the tail; the
    last store splits into two half-triggers so wire time overlaps the
    final drains.
  - psq drain order is (1,2,3,0) before a following projection (mt0 is
    double-buffered; mt1-3 drain first so the next s-chunk's kt0 hits
    them ready) and (0,1,2,3) before an out-stage (consumption order).
  - Kept from v4: xT as fp8 e3m4 (mixed f16xf8e3 matmul verified on HW:
    rel_err 1.354e-2 = exactly the numpy simulation; fp8 e4m3 anywhere
    in the projection FAILS the 2e-2 gate at 2.5-3.5e-2), e4m3 DoubleRow
    Gram, warmup chain + early ACT-table preload, drains split in halves
    across Vector+Activation, all input DMA on one Sync-ring in
    consumption order.

Tensor order: warmup | proj0 | proj1 | gram | out0 | proj2 | out1 |
out2 | proj3 | out3.

Device layout per core (all partition-outer):
    xT  [SC*128, KT*512] f8e3  xT[k, s] blocked: row sc*128+p holds all
                               kt chunks for that (sc, p)
    xn  [128, ST*512]    f8e4  row p holds all st chunks
    wT  [128, KT*512]    f16   1024*W_hg^T, row p = all kt chunks
    outB [128, SC*MT*512] f16  row p = out^T chunks; host reassembles
"""

import numpy as np

B, S, H = 4, 2048, 1024
N_HEADS = 16
HD = H // N_HEADS  # 64
N_CORES = 8
MG = H // 2        # 512 output columns per core
P = 128
KT = H // P        # 8 k-tiles
ST = S // P        # 16 s-tiles
MT = MG // P       # 4 m-tiles == head pairs
SC = S // 512      # 4 s-chunks
W_SCALE = 1024.0
N_WARMUP = 8

_NC_CACHE = {}


def _build_nc():
    import concourse.mybir as mybir
    from concourse import bacc
    from concourse.tile import TileContext

    f32 = mybir.dt.float32
    f16 = mybir.dt.float16
    f8e4 = mybir.dt.float8e4
    f8e3 = mybir.dt.float8e3
    DR = mybir.MatmulPerfMode.DoubleRow

    nc = bacc.Bacc()
    wT_d = nc.declare_dram_parameter("wT", [P, KT * MG], f16, isOutput=False)
    xT_d = nc.declare_dram_parameter("xT", [SC * P, KT * 512], f8e3, isOutput=False)
    xn_d = nc.declare_dram_parameter("xn", [P, ST * MG], f8e4, isOutput=False)
    outB_d = nc.declare_dram_parameter(
        "outB", [P, SC * MT * 512], f16, isOutput=True
    )

    wT_t = wT_d.rearrange("p (kt m) -> p kt m", kt=KT)           # [128, 8, 512]
    xT_t = xT_d.rearrange("(sc p) (kt n) -> p sc kt n", sc=SC, kt=KT)
    xn_t = xn_d.rearrange("p (st m) -> p st m", st=ST)           # [128, 16, 512]
    outB_t = outB_d.rearrange("p (sc mt n) -> p sc mt n", sc=SC, mt=MT)

    with TileContext(nc) as tc:
        with (
            tc.tile_pool(name="big", bufs=1) as big,
            tc.tile_pool(name="gp", bufs=1) as gpool,
            tc.tile_pool(name="stage", bufs=4) as stage,
            tc.tile_pool(name="ps_q0", bufs=2, space="PSUM") as ps_q0,
            tc.tile_pool(name="ps_q", bufs=1, space="PSUM") as ps_q,
            tc.tile_pool(name="ps_o", bufs=3, space="PSUM") as ps_o,
        ):
            wT_sb = big.tile([P, KT, MG], f16, tag="wT")
            xT_sb = big.tile([P, SC, KT, 512], f8e3, tag="xT")
            xn_sb = big.tile([P, ST, MG], f8e4, tag="xn")
            q_sb = big.tile([P, MT, S], f16, tag="q")

            # ---- Warmup: one back-to-back accumulation chain (same psum
            # tile, same engine => no semaphores) spins the PE p-state up
            # during the initial DMA latency window. The scalar-engine copy
            # forces the lazy ACT_TABLE_LOAD into this idle window too.
            wu_sb = gpool.tile([P, 512], f16, tag="wu", name="wu_sb")
            nc.vector.memset(wu_sb, 0.0)
            nc.scalar.copy(out=wu_sb[:, 256:264], in_=wu_sb[:, 0:8])
            gbd = []
            for p_i in range(MT):
                g = gpool.tile([P, P], f16, tag=f"g{p_i}", name=f"g{p_i}")
                nc.vector.memset(g, 0.0)
                gbd.append(g)
            wu_ps = ps_o.tile([P, 256], f32, tag="pso", name="wu_ps")
            for i in range(N_WARMUP):
                nc.tensor.matmul(
                    wu_ps,
                    lhsT=wu_sb[:, 0:P],
                    rhs=wu_sb[:, 0:256],
                    start=(i == 0),
                    stop=(i == N_WARMUP - 1),
                )

            # ---- Input DMA ring (Sync engine), exact consumption order,
            # >=192KiB per trigger after the first pair so the wire
            # (~300GB/s) outruns trigger issue (~0.65us each).
            nc.sync.dma_start(out=wT_sb[:, 0], in_=wT_t[:, 0])
            nc.sync.dma_start(out=xT_sb[:, 0, 0], in_=xT_t[:, 0, 0])
            for a, z in ((1, 3), (3, 5), (5, 7), (7, 8)):
                nc.sync.dma_start(out=wT_sb[:, a:z], in_=wT_t[:, a:z])
                nc.sync.dma_start(out=xT_sb[:, 0, a:z], in_=xT_t[:, 0, a:z])
            nc.sync.dma_start(out=xT_sb[:, 1, 0:4], in_=xT_t[:, 1, 0:4])
            nc.sync.dma_start(out=xT_sb[:, 1, 4:8], in_=xT_t[:, 1, 4:8])
            nc.sync.dma_start(out=xn_sb[:, 0:8], in_=xn_t[:, 0:8])
            nc.sync.dma_start(out=xn_sb[:, 8:16], in_=xn_t[:, 8:16])
            nc.sync.dma_start(out=xT_sb[:, 2], in_=xT_t[:, 2])
            nc.sync.dma_start(out=xT_sb[:, 3], in_=xT_t[:, 3])

            def proj(sc, drain_order):
                psqs = [
                    (ps_q0 if mt == 0 else ps_q).tile(
                        [P, 512], f32, tag=f"psq{mt}", name=f"psq{sc}_{mt}"
                    )
                    for mt in range(MT)
                ]
                for kt in range(KT):
                    for mt in range(MT):
                        nc.tensor.matmul(
                            psqs[mt],
                            lhsT=wT_sb[:, kt, mt * P:(mt + 1) * P],
                            rhs=xT_sb[:, sc, kt],
                            start=(kt == 0),
                            stop=(kt == KT - 1),
                        )
                for mt in drain_order:
                    lo = q_sb[:, mt, sc * 512:sc * 512 + 256]
                    hi = q_sb[:, mt, sc * 512 + 256:(sc + 1) * 512]
                    nc.vector.tensor_copy(out=lo, in_=psqs[mt][:, 0:256])
                    nc.scalar.copy(out=hi, in_=psqs[mt][:, 256:512])

            def gram():
                # Pair-Grams cycle the 3-deep pso ring; each pair's two
                # scale-copies run on DVE/Act while the next pair matmuls.
                for p_i in range(MT):
                    psg = ps_o.tile([P, P], f32, tag="pso", name=f"psg{p_i}")
                    xp = xn_sb[:, :, p_i * P:(p_i + 1) * P]
                    for i in range(ST // 2):
                        nc.tensor.matmul(
                            psg,
                            lhsT=xp[:, 2 * i:2 * i + 2],
                            rhs=xp[:, 2 * i:2 * i + 2],
                            start=(i == 0),
                            stop=(i == ST // 2 - 1),
                            perf_mode=DR,
                        )
                    nc.vector.tensor_scalar_mul(
                        out=gbd[p_i][0:HD, 0:HD],
                        in0=psg[0:HD, 0:HD],
                        scalar1=1.0 / W_SCALE,
                    )
                    nc.scalar.mul(
                        gbd[p_i][HD:P, HD:P], psg[HD:P, HD:P], 1.0 / W_SCALE
                    )

            def out_stage(sc):
                ot = stage.tile([P, MT, 512], f16, tag="ot", name=f"ot{sc}")
                for mt in range(MT):
                    pso = ps_o.tile([P, 512], f32, tag="pso", name=f"pso{sc}_{mt}")
                    nc.tensor.matmul(
                        pso,
                        lhsT=gbd[mt],
                        rhs=q_sb[:, mt, sc * 512:(sc + 1) * 512],
                        start=True,
                        stop=True,
                    )
                    nc.vector.tensor_copy(out=ot[:, mt, 0:256], in_=pso[:, 0:256])
                    nc.scalar.copy(out=ot[:, mt, 256:512], in_=pso[:, 256:512])
                    if sc == SC - 1:
                        # Per-mt stores so the last chunk's wire time
                        # overlaps the remaining drains.
                        nc.gpsimd.dma_start(
                            out=outB_t[:, sc, mt:mt + 1], in_=ot[:, mt:mt + 1]
                        )
                if sc != SC - 1:
                    nc.gpsimd.dma_start(out=outB_t[:, sc], in_=ot)

            proj(0, (1, 2, 3, 0))
            proj(1, (1, 2, 3, 0))
            gram()
            out_stage(0)
            proj(2, (0, 1, 2, 3))
            out_stage(1)
            out_stage(2)
            proj(3, (0, 1, 2, 3))
            out_stage(3)
    nc.compile()
    return nc


def _get_nc():
    if "nc" not in _NC_CACHE:
        _NC_CACHE["nc"] = _build_nc()
    return _NC_CACHE["nc"]


def make_in_maps(hidden_states, queries_weight):
    import ml_dtypes

    f8e4 = ml_dtypes.float8_e4m3
    f8e3 = ml_dtypes.float8_e3m4
    hs = np.ascontiguousarray(np.asarray(hidden_states, dtype=np.float32))
    w = np.ascontiguousarray(np.asarray(queries_weight, dtype=np.float32))
    in_maps = []
    for c in range(N_CORES):
        b, hg = divmod(c, 2)
        xb = hs[b]
        xT = np.ascontiguousarray(xb.T)  # [1024, 2048]
        in_maps.append({
            # row sc*128+p holds kt-major chunks: [SC*P, KT*512]
            "xT": np.ascontiguousarray(
                xT.reshape(KT, P, SC, 512).transpose(2, 1, 0, 3).reshape(
                    SC * P, KT * 512
                )
            ).astype(f8e3),
            # row p holds st-major chunks: [P, ST*MG]
            "xn": np.ascontiguousarray(
                xb[:, hg * MG:(hg + 1) * MG]
                .reshape(ST, P, MG).transpose(1, 0, 2).reshape(P, ST * MG)
            ).astype(f8e4),
            # row p holds kt-major chunks: [P, KT*MG]
            "wT": np.ascontiguousarray(
                (w[hg * MG:(hg + 1) * MG, :].T * W_SCALE)
                .reshape(KT, P, MG).transpose(1, 0, 2).reshape(P, KT * MG)
            ).astype(np.float16),
        })
    return in_maps


def assemble_output(results):
    out = np.empty((B, S, H), dtype=np.float32)
    for c in range(N_CORES):
        b, hg = divmod(c, 2)
        r = np.asarray(results[c]["outB"])  # [P, SC*MT*512] f16
        out[b, :, hg * MG:(hg + 1) * MG] = (
            r.reshape(P, SC, MT, 512).transpose(1, 3, 2, 0).reshape(S, MG)
        ).astype(np.float32)
    return out


def kernel(hidden_states, queries_weight):
    from concourse.bass_utils import run_bass_kernel_spmd

    in_maps = make_in_maps(hidden_states, queries_weight)
    res = run_bass_kernel_spmd(
        _get_nc(), in_maps, core_ids=list(range(N_CORES))
    ).results
    return assemble_output(res)


if __name__ == "__main__":
    x = np.random.randn(B, S, H).astype(np.float32)
    w = np.random.randn(H, H).astype(np.float32) * 1e-4
    out = kernel(x, w)
    print(out.shape, out.dtype)
